# revision 48
# baseline (speedup 1.0000x reference)
"""Trainium2 kernel for nn_AMPSShare (AMPS log-likelihood).

Math
----
The reference computes the log-likelihood of binary strings under an
autoregressive MPS with per-site matrices A[i,:,:,s] = I + t[i,:,:,s],
where t = `tensors` input with std 1e-8.  Per step i the contribution
reduces exactly (log-softmax of 2 logits) to

    contrib_i(b) = x_i(b) * Yd_i(b) - softplus(Yd_i(b)),
    Yd_i(b)      = lv_{i-1}(b) . (A_i0 - A_i1)[:, 0],

and lv deviates from e_0 only at O(n * 1e-8) ~ 1e-5, making
Yd_i(b) = t[i,0,0,0] - t[i,0,0,1] + O(1e-13)  (batch independent).
Hence, to far below f32 resolution,

    out(b) = c + sum_n data[b, n] * yd[n],
    yd[n]  = tensors[n,0,0,0] - tensors[n,0,0,1],
    c      = -sum_n softplus(yd[n]).

This is a pure data-parallel matvec over the 51 MB `data` tensor: the
memory-roofline computation for this problem.  A guard falls back to the
exact sequential recurrence (host) if `tensors` is ever not small.

Device mapping (8 NeuronCores, batch-sharded 2048 rows/core)
------------------------------------------------------------
Rows 0..14 of the 6.27 MB shard stream via SWDGE cast-DMAs (f32 DRAM
-> bf16 SBUF; values are {0,1} so the cast is exact) issued by the
otherwise idle GpSimd engine - rows 0..13 full, row 14 in halves
(>=512 B/partition; bf16 quarters hit the sub-512B SDMA RMW path and
corrupt), ending in a dummy chunk so row 14's consumers can wait
one-behind for a drain of cast-write-visibility margin.  Row 15 rides
the sync HWDGE queue in f32 and lands early, so its whole
mul/partial-reduce chain runs mid-stream, off the tail.  The yd
broadcast is pre-cast to bf16 on host (0.4% rounding, ~1e9 below the
|out| ~ 543 scale).  bf16 inputs double the DVE element rate (477 vs
885 ns per row), so DVE does all products plus the row 13/14/15
reduces while ACT row-sums rows 0-12 via activation(Copy, accum_out);
the ACT function-table load (~1.3 us) is hoisted to block start by a
dummy activation on a const AP.  Products go to full-size buffers (no
reuse -> no WAR stalls); every data chunk gets its own completion
semaphore (16 incs = that chunk fully landed, immune to SDMA engine
skew - a shared counter let fast engines' increments mask a slow
engine's unfinished chunk, which corrupted row 15 on cold runs).
dsb15/stage/out_sb are zero-initialized at block start: every
observed first-run-after-compile glitch was cold-SBUF garbage read
through one of them, and zeros turn any residual window into a
sub-1e-6 error.  Output leaves in two DMAs (cols 0:10 mid-stream,
cols 10:16 at the tail); the softplus constant c is added on host
during unshard.

(Fused single-pass reduction is not available: the native
TENSOR_TENSOR_REDUCE isa op fails this image's neuronxcc with 'ISA
wrong length' - reconfirmed - and TensorScalarPtr with accum_out
computes corrupted row sums on HW when ops run back-to-back despite
exact CoreSim results.  A TensorE ones x yd broadcast of a [1,784]
aux also NaN'd on HW.  tensor_tensor_scan cannot express a
multiply-accumulate recurrence, and tensor_reduce runs at ~1
elem/cycle regardless of dtype, so mul+reduce on two engines is the
minimum op mix.)
"""

import contextlib
import sys

import numpy as np

if "/opt/trn_rl_repo" not in sys.path:
    sys.path.insert(0, "/opt/trn_rl_repo")

N = 784
BS = 16384
NCORES = 8
SHARD = BS // NCORES          # 2048 rows per core
P = 128                       # SBUF partitions
T = SHARD // P                # 16 batch rows per partition
H = N // 2                    # 392: half-row split (row 14)
Q = N // 4                    # 196: quarter-row split (row 15)

_CACHE = {}

# rows whose reduction runs on DVE (products consumed by tensor_reduce);
# ACT accumulates the rest (in its program order)
DVE_RED_ROWS = (13, 14)
ACT_ROWS = (0, 1, 2, 3, 4, 5, 6, 7, 8, 9, 10, 11, 12)


def _build_nc():
    import concourse.bass as bass
    from concourse import mybir

    f32 = mybir.dt.float32
    bf16 = mybir.dt.bfloat16
    nc = bass.Bass(enable_partition_id=False)
    data = nc.declare_dram_parameter("data", [SHARD, N], f32, isOutput=False)
    aux = nc.declare_dram_parameter("aux", [P, N], bf16, isOutput=False)
    out = nc.declare_dram_parameter("out", [P, T], f32, isOutput=True)

    dview = data[:].rearrange("(p t) n -> p t n", t=T)
    copy = mybir.ActivationFunctionType.Copy
    red_add = mybir.AluOpType.add
    ax_x = mybir.AxisListType.X

    # SWDGE cast-DMA chunks (row0, row1, col0, col1) over rows [row0,row1),
    # in land order: rows 0,1 single (feed ACT's rate chain ASAP), rows
    # 2..13 as pair-row DMAs (halves Q7 descriptor-emission and ring
    # pressure), then row 14 in halves (>=512 B/partition: bf16 quarters
    # would take the sub-512B SDMA RMW path, which corrupted writes on HW).
    # Row 15 does NOT ride this queue: cold-HW runs showed the LAST chunks
    # of the SWDGE cast queue can have their completion increments land
    # before the cast-writes are visible, so the final row goes over the
    # proven HWDGE/f32 sync queue instead (landing early, off the tail),
    # and the queue ends with a dummy chunk so row 14's muls can wait
    # one-behind for a full drain of visibility margin.
    chunks = [(0, 1, 0, N), (1, 2, 0, N)]
    chunks += [(t, t + 2, 0, N) for t in range(2, T - 2, 2)]
    chunks += [(T - 2, T - 1, 0, H), (T - 2, T - 1, H, N)]
    NCHUNK = len(chunks)      # 10
    NLANE = 4

    # chunk index covering each row
    last_chunk_of_row = {}
    for k, (t0, t1, c0, c1) in enumerate(chunks):
        for t in range(t0, t1):
            last_chunk_of_row[t] = k


    # DVE program order bookkeeping (psem values)
    # muls: chunks 0..19 -> psem 1..20 interleaved with reds below
    with (
        nc.sbuf_tensor([P, T - 1, N], bf16) as dsb,
        nc.sbuf_tensor([P, N], f32) as dsb15,     # row 15, HWDGE f32 path
        nc.sbuf_tensor([P, N], bf16) as aux_sb,
        nc.sbuf_tensor([P, N], f32) as aux32,     # f32 copy for row 15
        nc.sbuf_tensor([P, T, N], bf16) as prods,
        nc.sbuf_tensor([P, N], f32) as prods15,
        nc.sbuf_tensor([P, 128], f32) as scrap,   # dummy-chunk landing pad
        nc.sbuf_tensor([P, 4], f32) as stage,     # partial-reduce staging
        nc.sbuf_tensor([P, 2], f32) as dump_act,
        nc.sbuf_tensor([P, T], f32) as out_sb,
        contextlib.ExitStack() as stack,
        nc.semaphore() as asem,   # aux DMA
        nc.semaphore() as f1sem,  # row-15 first-half f32 DMA
        nc.semaphore() as f2sem,  # row-15 second-half f32 DMA
        nc.semaphore() as psem,   # DVE ops, +1 each
        nc.semaphore() as ssem,   # ACT ops, +1 each
        nc.Block() as blk,
    ):
        # one completion semaphore per data chunk (+1 for the trailing
        # dummy): 16 increments (one per SDMA engine) mean THAT chunk
        # fully landed, immune to engine skew
        dsems = [
            stack.enter_context(nc.semaphore(f"dsem{k}"))
            for k in range(NCHUNK + 1)
        ]

        # DVE op sequence: (kind, payload); psem increments by 1 per op.
        # Rows 0-1 first (feed ACT's rate chain), then the whole row-15
        # f32 chain early (its data lands ~10us on the sync queue), then
        # rows 2..14 with DVE reduces interleaved; row 14's reduce runs as
        # two half-partials so the post-stream tail ends in a 60ns combine.
        dve_ops = []
        psem_val = {}

        def emit(op):
            dve_ops.append(op)
            psem_val[op[0] if op[0] != "mul" else ("mul", op[1][1])] = len(dve_ops)

        # defensive zero-init first (cold-SBUF garbage in these buffers is
        # what every observed first-run glitch read); psem-gated so the
        # dsb15 DMAs only start after its memset
        emit(("ms_dsb15", None))
        emit(("ms_misc", None))
        emit(("mul", (0, 0, 0, N)))
        emit(("mul", (1, 1, 0, N)))
        emit(("copy32", None))
        emit(("mul15a", None))
        emit(("red15a", None))
        emit(("mul15b", None))
        emit(("red15b", None))
        emit(("add2", None))
        for k in range(2, NCHUNK):
            t0, t1, c0, c1 = chunks[k]
            for t in range(t0, t1):
                emit(("mul", (k, t, c0, c1)))
                if t in DVE_RED_ROWS and t != T - 2 and c1 == N:
                    emit(("red_row%d" % t, t))
            if t0 == T - 2:
                # row-14 half-partials right after each half's mul
                emit(("red14a" if c0 == 0 else "red14b", c0))
        emit(("add2_14", None))
        NDVE = len(dve_ops)

        @blk.sync
        def _(s):
            s.dma_start(out=aux_sb[:], in_=aux[:]).then_inc(asem, 16)
            s.wait_ge(psem, 1)  # dsb15 zero-init done
            s.dma_start(out=dsb15[:, 0:H], in_=dview[:, T - 1, 0:H]).then_inc(
                f1sem, 16
            )
            s.dma_start(out=dsb15[:, H:N], in_=dview[:, T - 1, H:N]).then_inc(
                f2sem, 16
            )
            # out cols 0:10 once act10's inc implies READ_ACC rows <= 9
            s.wait_ge(ssem, 11)
            s.dma_start(out=out[:, 0:10], in_=out_sb[:, 0:10]).then_inc(
                asem, 16
            )
            # marker: all ACT readouts committed; psem NDVE: all DVE done
            s.wait_ge(ssem, len(ACT_ROWS) + 1)
            s.wait_ge(psem, NDVE)
            s.dma_start(out=out[:, 10:T], in_=out_sb[:, 10:T]).then_inc(
                asem, 16
            )

        @blk.gpsimd
        def _(g):
            for k, (t0, t1, c0, c1) in enumerate(chunks):
                g.dma_start(
                    out=dsb[:, t0:t1, c0:c1], in_=dview[:, t0:t1, c0:c1]
                ).then_inc(dsems[k], 16)
            # trailing dummy: one more queue entry so row 14's consumers can
            # wait one-behind (512 B/partition keeps it off the RMW path)
            g.dma_start(
                out=scrap[:], in_=dview[:, T - 2, 0:128]
            ).then_inc(dsems[NCHUNK], 16)

        @blk.scalar
        def _(a):
            # dummy: hoist the ACT function-table load off the critical path
            nc.scalar.activation(
                dump_act[:, 0:1], nc.const_aps.tensor(0.0, (P, 1), f32), copy
            )
            for t in ACT_ROWS:
                a.wait_ge(psem, psem_val[("mul", t)])
                nc.scalar.activation(
                    dump_act[:, 0:1].broadcast_to((P, N)),
                    prods[:, t, :],
                    copy,
                    accum_out=out_sb[:, t : t + 1],
                ).then_inc(ssem, 1)    # ssem 1..12
            # marker: sequences after the last READ_ACC
            nc.scalar.activation(
                dump_act[:, 0:1], out_sb[:, ACT_ROWS[-1] : ACT_ROWS[-1] + 1], copy
            ).then_inc(ssem, 1)

        @blk.vector
        def _(v):
            for kind, payload in dve_ops:
                if kind == "ms_dsb15":
                    nc.vector.memset(dsb15[:], 0.0).then_inc(psem, 1)
                    continue
                if kind == "ms_misc":
                    nc.vector.memset(stage[:], 0.0)
                    nc.vector.memset(out_sb[:], 0.0).then_inc(psem, 1)
                    v.wait_ge(asem, 16)
                    continue
                if kind == "mul":
                    k, t, c0, c1 = payload
                    v.wait_ge(dsems[k], 16)
                    if t == T - 2:
                        # one-behind: also require the NEXT queue entry so
                        # the cast-writes have a full drain of margin
                        v.wait_ge(dsems[k + 1], 16)
                    nc.vector.tensor_mul(
                        prods[:, t, c0:c1], dsb[:, t, c0:c1], aux_sb[:, c0:c1]
                    ).then_inc(psem, 1)
                elif kind == "copy32":
                    nc.vector.tensor_copy(aux32[:], aux_sb[:]).then_inc(psem, 1)
                elif kind == "mul15a":
                    v.wait_ge(f1sem, 16)
                    nc.vector.tensor_mul(
                        prods15[:, 0:H], dsb15[:, 0:H], aux32[:, 0:H]
                    ).then_inc(psem, 1)
                elif kind == "mul15b":
                    v.wait_ge(f2sem, 16)
                    nc.vector.tensor_mul(
                        prods15[:, H:N], dsb15[:, H:N], aux32[:, H:N]
                    ).then_inc(psem, 1)
                elif kind == "red15a":
                    nc.vector.tensor_reduce(
                        stage[:, 0:1], prods15[:, 0:H], ax_x, red_add
                    ).then_inc(psem, 1)
                elif kind == "red15b":
                    nc.vector.tensor_reduce(
                        stage[:, 1:2], prods15[:, H:N], ax_x, red_add
                    ).then_inc(psem, 1)
                elif kind == "add2":
                    nc.vector.tensor_reduce(
                        out_sb[:, T - 1 : T], stage[:, 0:2], ax_x, red_add
                    ).then_inc(psem, 1)
                elif kind == "red14a":
                    nc.vector.tensor_reduce(
                        stage[:, 2:3], prods[:, T - 2, 0:H], ax_x, red_add
                    ).then_inc(psem, 1)
                elif kind == "red14b":
                    nc.vector.tensor_reduce(
                        stage[:, 3:4], prods[:, T - 2, H:N], ax_x, red_add
                    ).then_inc(psem, 1)
                elif kind == "add2_14":
                    nc.vector.tensor_reduce(
                        out_sb[:, T - 2 : T - 1], stage[:, 2:4], ax_x, red_add
                    ).then_inc(psem, 1)
                else:  # red_row13
                    t = payload
                    nc.vector.tensor_reduce(
                        out_sb[:, t : t + 1], prods[:, t, :], ax_x, red_add
                    ).then_inc(psem, 1)

    return nc


def _get_nc():
    if "nc" not in _CACHE:
        _CACHE["nc"] = _build_nc()
    return _CACHE["nc"]


def _device_matvec(data, aux, trace=False, **kw):
    from concourse.bass_utils import run_bass_kernel_spmd

    nc = _get_nc()
    in_maps = [
        {"data": np.ascontiguousarray(data[c * SHARD : (c + 1) * SHARD]), "aux": aux}
        for c in range(NCORES)
    ]
    res = run_bass_kernel_spmd(
        nc, in_maps, core_ids=list(range(NCORES)), trace=trace, **kw
    )
    out = np.concatenate([res.results[c]["out"].reshape(SHARD) for c in range(NCORES)])
    return out, res


def _host_exact(data, tensors):
    """Exact recurrence in float64 on host; fallback only (never expected
    for this problem's input distribution)."""
    d = data.astype(np.float64)
    t = tensors.astype(np.float64)
    eye = np.eye(t.shape[1])
    A0 = t[:, :, :, 0] + eye
    A1 = t[:, :, :, 1] + eye
    bs, n = d.shape
    out = np.zeros(bs)
    u = np.stack([np.full(bs, A0[0, 0, 0]), np.full(bs, A1[0, 0, 0])], axis=1)
    lv = A1[0, 0][None, :] + d[:, 0:1] * (A0[0, 0] - A1[0, 0])[None, :]
    m = u.max(axis=1)
    lse = m + np.log(np.exp(u[:, 0] - m) + np.exp(u[:, 1] - m))
    out += d[:, 0] * u[:, 0] + (1 - d[:, 0]) * u[:, 1] - lse
    for i in range(1, n):
        u0 = lv @ A0[i, :, 0]
        u1 = lv @ A1[i, :, 0]
        m = np.maximum(u0, u1)
        lse = m + np.log(np.exp(u0 - m) + np.exp(u1 - m))
        out += d[:, i] * u0 + (1 - d[:, i]) * u1 - lse
        lv = lv @ A1[i] + d[:, i : i + 1] * (lv @ (A0[i] - A1[i]))
    return out.astype(np.float32)


def _make_aux(tensors):
    """yd row pre-broadcast to (P, N) bf16 plus the softplus constant c."""
    import ml_dtypes

    t64 = tensors.astype(np.float64)
    yd = t64[:, 0, 0, 0] - t64[:, 0, 0, 1]
    c = -np.sum(np.log1p(np.exp(yd)))
    aux = np.ascontiguousarray(
        np.broadcast_to(yd.astype(ml_dtypes.bfloat16)[None, :], (P, N))
    )
    return aux, np.float32(c)


def kernel(data, tensors):
    data = np.asarray(data, dtype=np.float32)
    tensors = np.asarray(tensors, dtype=np.float32)
    if np.abs(tensors).max() > 1e-3:
        # linearization invalid for large perturbations
        return _host_exact(data, tensors)
    aux, c = _make_aux(tensors)
    try:
        out, _ = _device_matvec(data, aux)
    except Exception as e:  # device unavailable: keep the answer correct
        print(f"kernel: device path failed ({e!r}); host fallback", file=sys.stderr)
        out = data @ aux[0].astype(np.float32)
    return (out + c).astype(np.float32)


def kernel_profiled(data, tensors, **kw):
    """Same as kernel() but with neuron-profile tracing; returns
    (output, BassKernelResults with exec_time_ns)."""
    data = np.asarray(data, dtype=np.float32)
    tensors = np.asarray(tensors, dtype=np.float32)
    aux, c = _make_aux(tensors)
    out, res = _device_matvec(data, aux, trace=True, **kw)
    return (out + c).astype(np.float32), res


# revision 55
# speedup vs baseline: 1.0443x; 1.0443x over previous
"""Trainium2 kernel for nn_AMPSShare (AMPS log-likelihood).

Math
----
The reference computes the log-likelihood of binary strings under an
autoregressive MPS with per-site matrices A[i,:,:,s] = I + t[i,:,:,s],
where t = `tensors` input with std 1e-8.  Per step i the contribution
reduces exactly (log-softmax of 2 logits) to

    contrib_i(b) = x_i(b) * Yd_i(b) - softplus(Yd_i(b)),
    Yd_i(b)      = lv_{i-1}(b) . (A_i0 - A_i1)[:, 0],

and lv deviates from e_0 only at O(n * 1e-8) ~ 1e-5, making
Yd_i(b) = t[i,0,0,0] - t[i,0,0,1] + O(1e-13)  (batch independent).
Hence, to far below f32 resolution,

    out(b) = c + sum_n data[b, n] * yd[n],
    yd[n]  = tensors[n,0,0,0] - tensors[n,0,0,1],
    c      = -sum_n softplus(yd[n]).

This is a pure data-parallel matvec over the 51 MB `data` tensor: the
memory-roofline computation for this problem.  A guard falls back to the
exact sequential recurrence (host) if `tensors` is ever not small.

Device mapping (8 NeuronCores, batch-sharded 2048 rows/core)
------------------------------------------------------------
Rows 0..14 of the 6.27 MB shard stream via SWDGE cast-DMAs (f32 DRAM
-> bf16 SBUF; values are {0,1} so the cast is exact) issued by the
otherwise idle GpSimd engine - rows 0..13 full, row 14 in halves
(>=512 B/partition; bf16 quarters hit the sub-512B SDMA RMW path and
corrupt), ending in a dummy chunk so row 14's consumers can wait
one-behind for a drain of cast-write-visibility margin.  Row 15 rides
the sync HWDGE queue in f32 and lands early, so its whole
mul/partial-reduce chain runs mid-stream, off the tail.  The yd
broadcast is pre-cast to bf16 on host (0.4% rounding, ~1e9 below the
|out| ~ 543 scale).  bf16 inputs double the DVE element rate (477 vs
885 ns per row), so DVE does all products plus the row 13/14/15
reduces while ACT row-sums rows 0-12 via activation(Copy, accum_out);
the ACT function-table load (~1.3 us) is hoisted to block start by a
dummy activation on a const AP.  Products go to full-size buffers (no
reuse -> no WAR stalls); every data chunk gets its own completion
semaphore (16 incs = that chunk fully landed, immune to SDMA engine
skew - a shared counter let fast engines' increments mask a slow
engine's unfinished chunk, which corrupted row 15 on cold runs).
dsb15/stage/out_sb are zero-initialized at block start: every
observed first-run-after-compile glitch was cold-SBUF garbage read
through one of them, and zeros turn any residual window into a
sub-1e-6 error.  Output leaves in two DMAs (cols 0:10 mid-stream,
cols 10:16 at the tail); the softplus constant c is added on host
during unshard.

(Fused single-pass reduction is not available: the native
TENSOR_TENSOR_REDUCE isa op fails this image's neuronxcc with 'ISA
wrong length' - reconfirmed - and TensorScalarPtr with accum_out
computes corrupted row sums on HW when ops run back-to-back despite
exact CoreSim results.  A TensorE ones x yd broadcast of a [1,784]
aux also NaN'd on HW.  tensor_tensor_scan cannot express a
multiply-accumulate recurrence, and tensor_reduce runs at ~1
elem/cycle regardless of dtype, so mul+reduce on two engines is the
minimum op mix.)
"""

import contextlib
import sys

import numpy as np

if "/opt/trn_rl_repo" not in sys.path:
    sys.path.insert(0, "/opt/trn_rl_repo")

N = 784
BS = 16384
NCORES = 8
SHARD = BS // NCORES          # 2048 rows per core
P = 128                       # SBUF partitions
T = SHARD // P                # 16 batch rows per partition
H = N // 2                    # 392: half-row split (row 14)
Q = N // 4                    # 196: quarter-row split (row 15)

_CACHE = {}

# rows whose reduction runs on DVE (products consumed by tensor_reduce);
# ACT accumulates the rest (in its program order)
DVE_RED_ROWS = (13, 14)
ACT_ROWS = (0, 1, 2, 3, 4, 5, 6, 7, 8, 9, 10, 11, 12)


def _build_nc():
    import concourse.bass as bass
    from concourse import mybir

    f32 = mybir.dt.float32
    bf16 = mybir.dt.bfloat16
    nc = bass.Bass(enable_partition_id=False)
    data = nc.declare_dram_parameter("data", [SHARD, N], f32, isOutput=False)
    aux = nc.declare_dram_parameter("aux", [P, N], bf16, isOutput=False)
    out = nc.declare_dram_parameter("out", [P, T], f32, isOutput=True)

    dview = data[:].rearrange("(p t) n -> p t n", t=T)
    copy = mybir.ActivationFunctionType.Copy
    red_add = mybir.AluOpType.add
    ax_x = mybir.AxisListType.X

    # SWDGE cast-DMA chunks (row0, row1, col0, col1) over rows [row0,row1),
    # in land order: rows 0..13 as single-row DMAs (pair-row chunks were
    # tried and made stream jitter WORSE), then row 14 in halves (>=512
    # B/partition: bf16 quarters would take the sub-512B SDMA RMW path,
    # which corrupted writes on HW).
    # Row 15 does NOT ride this queue: cold-HW runs showed the LAST chunks
    # of the SWDGE cast queue can have their completion increments land
    # before the cast-writes are visible, so the final row goes over the
    # proven HWDGE/f32 sync queue instead (landing early, off the tail),
    # and the queue ends with a dummy chunk so row 14's muls can wait
    # one-behind for a full drain of visibility margin.
    # row 0 in halves so ACT's rate-bound chain starts as early as possible
    chunks = [(0, 1, 0, H), (0, 1, H, N)]
    chunks += [(t, t + 1, 0, N) for t in range(1, T - 2)]
    chunks += [(T - 2, T - 1, 0, H), (T - 2, T - 1, H, N)]
    NCHUNK = len(chunks)      # 17
    NLANE = 4

    # chunk index covering each row
    last_chunk_of_row = {}
    for k, (t0, t1, c0, c1) in enumerate(chunks):
        for t in range(t0, t1):
            last_chunk_of_row[t] = k


    # DVE program order bookkeeping (psem values)
    # muls: chunks 0..19 -> psem 1..20 interleaved with reds below
    with (
        nc.sbuf_tensor([P, T - 1, N], bf16) as dsb,
        nc.sbuf_tensor([P, N], f32) as dsb15,     # row 15, HWDGE f32 path
        nc.sbuf_tensor([P, N], bf16) as aux_sb,
        nc.sbuf_tensor([P, N], f32) as aux32,     # f32 copy for row 15
        nc.sbuf_tensor([P, T, N], bf16) as prods,
        nc.sbuf_tensor([P, N], f32) as prods15,
        nc.sbuf_tensor([P, 128], f32) as scrap,   # dummy-chunk landing pad
        nc.sbuf_tensor([P, 4], f32) as stage,     # partial-reduce staging
        nc.sbuf_tensor([P, 2], f32) as dump_act,
        nc.sbuf_tensor([P, T], f32) as out_sb,
        contextlib.ExitStack() as stack,
        nc.semaphore() as asem,   # aux first-half DMA
        nc.semaphore() as a2sem,  # aux second-half DMA
        nc.semaphore() as f1sem,  # row-15 first-half f32 DMA
        nc.semaphore() as f2sem,  # row-15 second-half f32 DMA
        nc.semaphore() as psem,   # DVE ops, +1 each
        nc.semaphore() as ssem,   # ACT ops, +1 each
        nc.Block() as blk,
    ):
        # one completion semaphore per data chunk (+1 for the trailing
        # dummy): 16 increments (one per SDMA engine) mean THAT chunk
        # fully landed, immune to engine skew
        dsems = [
            stack.enter_context(nc.semaphore(f"dsem{k}"))
            for k in range(NCHUNK + 1)
        ]

        # DVE op sequence: (kind, payload); psem increments by 1 per op.
        # Muls run in land order; the row-15 f32 chain (its data lands
        # ~10us on the sync queue) is INTERLEAVED one op per mul gap -
        # each gap holds ~0.4us of land-paced DVE idle, so a contiguous
        # 2.7us chain would starve ACT's slower rate chain, but spread
        # ops are absorbed for free.  Row 14's reduce runs as two
        # half-partials so the post-stream tail ends in a 60ns combine.
        dve_ops = []
        psem_val = {}

        def emit(op):
            dve_ops.append(op)
            psem_val[op[0] if op[0] != "mul" else ("mul", op[1][1])] = len(dve_ops)

        # defensive zero-init first (cold-SBUF garbage in these buffers is
        # what every observed first-run glitch read); psem-gated so the
        # dsb15 DMAs only start after its memset
        emit(("ms_dsb15", None))
        emit(("ms_misc", None))
        chain = ["copy32", "mul15a", "red15a", "mul15b", "red15b", "add2"]
        for k in range(NCHUNK):
            t0, t1, c0, c1 = chunks[k]
            for t in range(t0, t1):
                emit(("mul", (k, t, c0, c1)))
                if t in DVE_RED_ROWS and t != T - 2 and c1 == N:
                    emit(("red_row%d" % t, t))
            if k >= 2 and chain:
                # one row-15 chain op per mul gap, starting after row 1
                emit((chain.pop(0), None))
            if t0 == T - 2:
                # row-14 half-partials right after each half's mul
                emit(("red14a" if c0 == 0 else "red14b", c0))
        emit(("add2_14", None))
        NDVE = len(dve_ops)

        @blk.sync
        def _(s):
            # aux in halves so mul0a (and so ACT's act0) starts sooner
            s.dma_start(out=aux_sb[:, 0:H], in_=aux[:, 0:H]).then_inc(asem, 16)
            s.dma_start(out=aux_sb[:, H:N], in_=aux[:, H:N]).then_inc(a2sem, 16)
            s.wait_ge(psem, 1)  # dsb15 zero-init done
            s.dma_start(out=dsb15[:, 0:H], in_=dview[:, T - 1, 0:H]).then_inc(
                f1sem, 16
            )
            s.dma_start(out=dsb15[:, H:N], in_=dview[:, T - 1, H:N]).then_inc(
                f2sem, 16
            )
            # out cols 0:10 once act10's inc implies READ_ACC rows <= 9
            s.wait_ge(ssem, 11)
            s.dma_start(out=out[:, 0:10], in_=out_sb[:, 0:10]).then_inc(
                asem, 16
            )
            # marker: all ACT readouts committed; psem NDVE: all DVE done
            s.wait_ge(ssem, len(ACT_ROWS) + 1)
            s.wait_ge(psem, NDVE)
            s.dma_start(out=out[:, 10:T], in_=out_sb[:, 10:T]).then_inc(
                asem, 16
            )

        @blk.gpsimd
        def _(g):
            for k, (t0, t1, c0, c1) in enumerate(chunks):
                g.dma_start(
                    out=dsb[:, t0:t1, c0:c1], in_=dview[:, t0:t1, c0:c1]
                ).then_inc(dsems[k], 16)
            # trailing dummy: one more queue entry so row 14's consumers can
            # wait one-behind (512 B/partition keeps it off the RMW path)
            g.dma_start(
                out=scrap[:], in_=dview[:, T - 2, 0:128]
            ).then_inc(dsems[NCHUNK], 16)

        @blk.scalar
        def _(a):
            # dummy: hoist the ACT function-table load off the critical path
            nc.scalar.activation(
                dump_act[:, 0:1], nc.const_aps.tensor(0.0, (P, 1), f32), copy
            )
            for t in ACT_ROWS:
                a.wait_ge(psem, psem_val[("mul", t)])
                nc.scalar.activation(
                    dump_act[:, 0:1].broadcast_to((P, N)),
                    prods[:, t, :],
                    copy,
                    accum_out=out_sb[:, t : t + 1],
                ).then_inc(ssem, 1)    # ssem 1..12
            # marker: sequences after the last READ_ACC
            nc.scalar.activation(
                dump_act[:, 0:1], out_sb[:, ACT_ROWS[-1] : ACT_ROWS[-1] + 1], copy
            ).then_inc(ssem, 1)

        @blk.vector
        def _(v):
            for kind, payload in dve_ops:
                if kind == "ms_dsb15":
                    nc.vector.memset(dsb15[:], 0.0).then_inc(psem, 1)
                    continue
                if kind == "ms_misc":
                    nc.vector.memset(stage[:], 0.0)
                    nc.vector.memset(out_sb[:], 0.0).then_inc(psem, 1)
                    continue
                if kind == "mul":
                    k, t, c0, c1 = payload
                    if k == 0:
                        v.wait_ge(asem, 16)
                    elif k == 1:
                        v.wait_ge(a2sem, 16)
                    v.wait_ge(dsems[k], 16)
                    if t == T - 2:
                        # one-behind: also require the NEXT queue entry so
                        # the cast-writes have a full drain of margin
                        v.wait_ge(dsems[k + 1], 16)
                    nc.vector.tensor_mul(
                        prods[:, t, c0:c1], dsb[:, t, c0:c1], aux_sb[:, c0:c1]
                    ).then_inc(psem, 1)
                elif kind == "copy32":
                    nc.vector.tensor_copy(aux32[:], aux_sb[:]).then_inc(psem, 1)
                elif kind == "mul15a":
                    v.wait_ge(f1sem, 16)
                    nc.vector.tensor_mul(
                        prods15[:, 0:H], dsb15[:, 0:H], aux32[:, 0:H]
                    ).then_inc(psem, 1)
                elif kind == "mul15b":
                    v.wait_ge(f2sem, 16)
                    nc.vector.tensor_mul(
                        prods15[:, H:N], dsb15[:, H:N], aux32[:, H:N]
                    ).then_inc(psem, 1)
                elif kind == "red15a":
                    nc.vector.tensor_reduce(
                        stage[:, 0:1], prods15[:, 0:H], ax_x, red_add
                    ).then_inc(psem, 1)
                elif kind == "red15b":
                    nc.vector.tensor_reduce(
                        stage[:, 1:2], prods15[:, H:N], ax_x, red_add
                    ).then_inc(psem, 1)
                elif kind == "add2":
                    nc.vector.tensor_reduce(
                        out_sb[:, T - 1 : T], stage[:, 0:2], ax_x, red_add
                    ).then_inc(psem, 1)
                elif kind == "red14a":
                    nc.vector.tensor_reduce(
                        stage[:, 2:3], prods[:, T - 2, 0:H], ax_x, red_add
                    ).then_inc(psem, 1)
                elif kind == "red14b":
                    nc.vector.tensor_reduce(
                        stage[:, 3:4], prods[:, T - 2, H:N], ax_x, red_add
                    ).then_inc(psem, 1)
                elif kind == "add2_14":
                    nc.vector.tensor_reduce(
                        out_sb[:, T - 2 : T - 1], stage[:, 2:4], ax_x, red_add
                    ).then_inc(psem, 1)
                else:  # red_row13
                    t = payload
                    nc.vector.tensor_reduce(
                        out_sb[:, t : t + 1], prods[:, t, :], ax_x, red_add
                    ).then_inc(psem, 1)

    return nc


def _get_nc():
    if "nc" not in _CACHE:
        _CACHE["nc"] = _build_nc()
    return _CACHE["nc"]


def _device_matvec(data, aux, trace=False, **kw):
    from concourse.bass_utils import run_bass_kernel_spmd

    nc = _get_nc()
    in_maps = [
        {"data": np.ascontiguousarray(data[c * SHARD : (c + 1) * SHARD]), "aux": aux}
        for c in range(NCORES)
    ]
    res = run_bass_kernel_spmd(
        nc, in_maps, core_ids=list(range(NCORES)), trace=trace, **kw
    )
    out = np.concatenate([res.results[c]["out"].reshape(SHARD) for c in range(NCORES)])
    return out, res


def _host_exact(data, tensors):
    """Exact recurrence in float64 on host; fallback only (never expected
    for this problem's input distribution)."""
    d = data.astype(np.float64)
    t = tensors.astype(np.float64)
    eye = np.eye(t.shape[1])
    A0 = t[:, :, :, 0] + eye
    A1 = t[:, :, :, 1] + eye
    bs, n = d.shape
    out = np.zeros(bs)
    u = np.stack([np.full(bs, A0[0, 0, 0]), np.full(bs, A1[0, 0, 0])], axis=1)
    lv = A1[0, 0][None, :] + d[:, 0:1] * (A0[0, 0] - A1[0, 0])[None, :]
    m = u.max(axis=1)
    lse = m + np.log(np.exp(u[:, 0] - m) + np.exp(u[:, 1] - m))
    out += d[:, 0] * u[:, 0] + (1 - d[:, 0]) * u[:, 1] - lse
    for i in range(1, n):
        u0 = lv @ A0[i, :, 0]
        u1 = lv @ A1[i, :, 0]
        m = np.maximum(u0, u1)
        lse = m + np.log(np.exp(u0 - m) + np.exp(u1 - m))
        out += d[:, i] * u0 + (1 - d[:, i]) * u1 - lse
        lv = lv @ A1[i] + d[:, i : i + 1] * (lv @ (A0[i] - A1[i]))
    return out.astype(np.float32)


def _make_aux(tensors):
    """yd row pre-broadcast to (P, N) bf16 plus the softplus constant c."""
    import ml_dtypes

    t64 = tensors.astype(np.float64)
    yd = t64[:, 0, 0, 0] - t64[:, 0, 0, 1]
    c = -np.sum(np.log1p(np.exp(yd)))
    aux = np.ascontiguousarray(
        np.broadcast_to(yd.astype(ml_dtypes.bfloat16)[None, :], (P, N))
    )
    return aux, np.float32(c)


def kernel(data, tensors):
    data = np.asarray(data, dtype=np.float32)
    tensors = np.asarray(tensors, dtype=np.float32)
    if np.abs(tensors).max() > 1e-3:
        # linearization invalid for large perturbations
        return _host_exact(data, tensors)
    aux, c = _make_aux(tensors)
    try:
        out, _ = _device_matvec(data, aux)
    except Exception as e:  # device unavailable: keep the answer correct
        print(f"kernel: device path failed ({e!r}); host fallback", file=sys.stderr)
        out = data @ aux[0].astype(np.float32)
    return (out + c).astype(np.float32)


def kernel_profiled(data, tensors, **kw):
    """Same as kernel() but with neuron-profile tracing; returns
    (output, BassKernelResults with exec_time_ns)."""
    data = np.asarray(data, dtype=np.float32)
    tensors = np.asarray(tensors, dtype=np.float32)
    aux, c = _make_aux(tensors)
    out, res = _device_matvec(data, aux, trace=True, **kw)
    return (out + c).astype(np.float32), res
